# revision 31
# baseline (speedup 1.0000x reference)
"""CLUB loss kernel for Trainium2, 8 NeuronCores, data-parallel over batch.

Math (see reference): two MLPs over modal_a produce mu and logvar; the loss
needs only two scalars:
  lld   = -0.5/B * sum_{i,h} (mu-b)^2 * iv
  bound = lld + 0.5/B * ( sum_h E[mu^2]_h * T0_h - 2 sum_h E[mu]_h * T1_h + T2 )
where iv = exp(-logvar).  Everything reduces to per-feature batch sums
  S1 = sum mu, S2 = sum mu^2, T0 = sum iv, T1 = sum mu*iv, T2 = sum mu^2*iv,
  U = sum mu*b*iv, V = sum b^2*iv      (P = sum (mu-b)^2*iv = T2 - 2U + V)
which each core computes for its batch shard in one pass (no device
collectives -- the host combines 8 tiny [128,42] stat tensors).

Design (all constants measured on HW via loop-delta probes):
 * PE (the wall, ~37.7us/body): all four matmuls in fp8 e4m3 DoubleRow
   (256-row contraction/instruction, 157 TF/s = 2x bf16), redundant
   ldweights deduped (72 loads left).  Tolerance is 2e-2; fp8 end-to-end
   error measures ~1e-4 (quantization noise cancels over 6.3M samples).
 * ACT (~32us): tanh(L1v) tanh(L1m) tanh(L2v) drains + Square(psum,+bias)
   for S2 + exp(-lv) for iv (accum T0).  All five funcs live in the single
   exp_and_others table set -- no table reloads.
 * DVE (~34us): psum drain mu=raw+b2m (tensor_scalar, accum S1; the one
   unavoidably-1x psum op) + fused scalar_tensor_tensor product+accum ops
   for T1/T2/U/V (~1.14us each on HW; the "fast" 2x/4x DVE modes of the
   cost model do NOT materialize on HW for stt/ts, so fused stt wins).
 * Schedule: L1v,L1m phases (L2 contracts the full hidden dim), then L2
   with psum-freeing ops (tanh/Square/drain) prompt and the stt stats
   lagged 2 tiles + tailed, spilling into the next body's DVE-idle L1
   window; h1v/h1m double-buffered so bodies pipeline across the For_i
   loop; mu/lv/iv/f 6-deep so the stats tail never blocks a drain.
 * GPSIMD only ships the single [128,42] stats DMA per body (its
   elementwise ops are far too slow on HW).
 * b^2 is a host-side input (bT2); P is recovered on the host in f64.
"""

import numpy as np
import ml_dtypes

import concourse.bacc as bacc
import concourse.tile as tile
import concourse.mybir as mybir
from concourse.bass_utils import run_bass_kernel_spmd

B, H = 8192, 768
NCORES = 8
BS = B // NCORES          # 1024 rows per core
P = 128
KT = H // P               # 6 contraction tiles of 128
KT2 = KT // 2             # 3 DoubleRow contraction tiles of 256
JT = H // P               # 6 output-feature tiles
NI = 2                    # 512-wide moving chunks per matmul
IC = BS // NI             # 512
NST = 7                   # stat columns per j tile

F32 = mybir.dt.float32
BF16 = mybir.dt.bfloat16
F8 = mybir.dt.float8e4
AF = mybir.ActivationFunctionType
ALU = mybir.AluOpType
DR = mybir.MatmulPerfMode.DoubleRow

_BF16 = ml_dtypes.bfloat16
_F8 = ml_dtypes.float8_e4m3

_CACHE = {}
DEDUP_LDW = True
FINE_HEAD = True

# stat column ids within a j group
C_T0, C_S1, C_T1, C_T2, C_V, C_U, C_S2 = range(7)


def _build(repeat=1, trace_sim=False, loop_n=None):
    nc = bacc.Bacc(trn_type="TRN2")

    aT_d = nc.dram_tensor("aT", [H, BS], F8, kind="ExternalInput")
    bT_d = nc.dram_tensor("bT", [H, BS], BF16, kind="ExternalInput")
    bT2_d = nc.dram_tensor("bT2", [H, BS], BF16, kind="ExternalInput")
    w_d = {
        name: nc.dram_tensor(name, [H, H], F8, kind="ExternalInput")
        for name in ("w1vT", "w2vT", "w1mT", "w2mT")
    }
    bias_d = nc.dram_tensor("biases", [P, 4 * JT], F32, kind="ExternalInput")
    stats_d = nc.dram_tensor("stats", [P, JT * NST], F32,
                             kind="ExternalOutput")

    with tile.TileContext(nc, trace_sim=trace_sim) as tc:
        with (
            tc.tile_pool(name="weights", bufs=1) as wp,
            tc.tile_pool(name="acts", bufs=1) as ap,
            tc.tile_pool(name="rot", bufs=3) as rot,
            tc.tile_pool(name="h1p", bufs=3) as h1p,
            tc.tile_pool(name="stat", bufs=3) as stp,
            tc.tile_pool(name="psum", bufs=4, space="PSUM") as pp,
        ):
            # --- persistent SBUF tensors -------------------------------------
            w_sb = {}
            for name in ("w1vT", "w1mT", "w2vT", "w2mT"):
                w_sb[name] = wp.tile([P, KT, H], F8, name=f"{name}_sb")
            aT_sb = ap.tile([P, KT, BS], F8, name="aT_sb")
            bT_sb = ap.tile([P, JT, BS], BF16, name="bT_sb")
            bT2_sb = ap.tile([P, JT, BS], BF16, name="bT2_sb")
            bias_sb = ap.tile([P, 4 * JT], F32, name="bias_sb")

            # --- input DMAs (phase order; interleave w1v/aT so the first
            # matmul's operands land first) -----------------------------------
            if FINE_HEAD:
                nc.sync.dma_start(w_sb["w1vT"][:, 0, :], w_d["w1vT"][0:P, :])
                nc.sync.dma_start(aT_sb[:, 0, 0:IC], aT_d[0:P, 0:IC])
                nc.sync.dma_start(bias_sb, bias_d[:, :])
                nc.sync.dma_start(aT_sb[:, 0, IC:BS], aT_d[0:P, IC:BS])
                rng0 = 1
            else:
                nc.sync.dma_start(bias_sb, bias_d[:, :])
                rng0 = 0
            for kt in range(rng0, KT):
                nc.sync.dma_start(w_sb["w1vT"][:, kt, :],
                                  w_d["w1vT"][kt * P:(kt + 1) * P, :])
                nc.sync.dma_start(aT_sb[:, kt, :],
                                  aT_d[kt * P:(kt + 1) * P, :])
            for kt in range(KT):
                nc.sync.dma_start(w_sb["w1mT"][:, kt, :],
                                  w_d["w1mT"][kt * P:(kt + 1) * P, :])
            for kt in range(KT):
                nc.sync.dma_start(w_sb["w2vT"][:, kt, :],
                                  w_d["w2vT"][kt * P:(kt + 1) * P, :])
            for kt in range(KT):
                nc.sync.dma_start(w_sb["w2mT"][:, kt, :],
                                  w_d["w2mT"][kt * P:(kt + 1) * P, :])
            for j in range(JT):
                nc.sync.dma_start(bT_sb[:, j, :], bT_d[j * P:(j + 1) * P, :])
            for j in range(JT):
                nc.sync.dma_start(bT2_sb[:, j, :],
                                  bT2_d[j * P:(j + 1) * P, :])

            def matmul_tile(w, rhs_sb, j):
                """768-deep fp8 DoubleRow matmul for feature tile j."""
                ps = pp.tile([P, BS], F32, tag="ps", name="ps")
                for t in range(KT2):
                    lhsT = w[:, 2 * t:2 * t + 2, j * P:(j + 1) * P]
                    for ic in range(NI):
                        nc.tensor.matmul(
                            ps[:, ic * IC:(ic + 1) * IC], lhsT,
                            rhs_sb[:, 2 * t:2 * t + 2, ic * IC:(ic + 1) * IC],
                            start=(t == 0), stop=(t == KT2 - 1),
                            perf_mode=DR)
                return ps

            def act_tanh(ps, out_sb, j, bias_col, accum=None):
                nc.scalar.activation(
                    out_sb[:, j, :], ps, AF.Tanh,
                    bias=bias_sb[:, bias_col: bias_col + 1],
                    accum_out=accum)

            def body():
                # per-body stats accumulator (one DMA at the end)
                st = stp.tile([P, JT * NST], F32, tag="st", name="st")
                # multi-buffered hidden activations: the next body's L1
                # writes another slot while this body's L2 still reads this
                # one, letting bodies pipeline across the For_i loop
                h1v_sb = h1p.tile([P, JT, BS], F8, tag="h1v", name="h1v_sb")
                h1m_sb = h1p.tile([P, JT, BS], F8, tag="h1m", name="h1m_sb")

                def sc(j, c):
                    return st[:, j * NST + c: j * NST + c + 1]

                def l1v(j):
                    ps = matmul_tile(w_sb["w1vT"], aT_sb, j)
                    act_tanh(ps, h1v_sb, j, 0 * JT + j)

                def l1m(j):
                    ps = matmul_tile(w_sb["w1mT"], aT_sb, j)
                    act_tanh(ps, h1m_sb, j, 2 * JT + j)

                def l2_head(j):
                    """Matmuls + the psum-freeing ops only: tanh(lv) on ACT
                    for the L2v psum; Square (S2 accum, ACT) and the
                    tensor_scalar drain (S1 accum, DVE) for the L2m psum.
                    The stt stats lag so drains stay prompt."""
                    ps = matmul_tile(w_sb["w2vT"], h1v_sb, j)
                    lv = rot.tile([P, BS], BF16, tag="lv", bufs=6)
                    nc.scalar.activation(
                        lv, ps, AF.Tanh,
                        bias=bias_sb[:, 1 * JT + j: 1 * JT + j + 1])
                    ps2 = matmul_tile(w_sb["w2mT"], h1m_sb, j)
                    bcol = bias_sb[:, 3 * JT + j: 3 * JT + j + 1]
                    s2 = rot.tile([P, BS], BF16, tag="s2")
                    nc.scalar.activation(s2, ps2, AF.Square, bias=bcol,
                                         accum_out=sc(j, C_S2))
                    mu = rot.tile([P, BS], BF16, tag="mu", bufs=6)
                    nc.vector.tensor_scalar(
                        mu, ps2, bcol, None,
                        ALU.add, op1=ALU.add, accum_out=sc(j, C_S1))
                    return mu, lv

                def stats_a(j, mu, lv):
                    """exp + the f/T1 stt -- dosed between drains (lag 2)."""
                    iv = rot.tile([P, BS], BF16, tag="iv", bufs=6)
                    nc.scalar.activation(iv, lv, AF.Exp, scale=-1.0,
                                         accum_out=sc(j, C_T0))
                    f = rot.tile([P, BS], BF16, tag="f", bufs=6)
                    nc.vector.scalar_tensor_tensor(
                        f, mu, 1.0, iv, ALU.bypass, ALU.mult,
                        accum_out=sc(j, C_T1))
                    return iv, f

                def stats_b(j, mu, iv, f):
                    """the T2/U/V stts -- pure SBUF tail work, issued after
                    all drains so it spills into the next body's L1 window
                    where DVE is otherwise idle."""
                    t2 = rot.tile([P, BS], BF16, tag="t2")
                    nc.vector.scalar_tensor_tensor(
                        t2, f, 1.0, mu, ALU.bypass, ALU.mult,
                        accum_out=sc(j, C_T2))
                    u = rot.tile([P, BS], BF16, tag="u")
                    nc.vector.scalar_tensor_tensor(
                        u, f, 1.0, bT_sb[:, j, :], ALU.bypass, ALU.mult,
                        accum_out=sc(j, C_U))
                    v = rot.tile([P, BS], BF16, tag="v")
                    nc.vector.scalar_tensor_tensor(
                        v, bT2_sb[:, j, :], 1.0, iv, ALU.bypass, ALU.mult,
                        accum_out=sc(j, C_V))

                # --- L1v, L1m phases (L2 contracts the FULL hidden dim,
                # so all of h1v/h1m must exist first), then interleaved L2
                # with the stt stats lagged 2 tiles + tailed, spilling into
                # the next body's DVE-idle L1 window; h1 double-buffering
                # lets bodies pipeline across the For_i loop.
                for j in range(JT):
                    l1v(j)
                for j in range(JT):
                    l1m(j)
                mi = {}
                ivf = {}
                for j in range(JT):
                    mi[j] = l2_head(j)
                    if j >= 2:
                        mu, lv = mi[j - 2]
                        ivf[j - 2] = stats_a(j - 2, mu, lv)
                for j in (JT - 2, JT - 1):
                    mu, lv = mi[j]
                    ivf[j] = stats_a(j, mu, lv)
                for j in range(JT):
                    mu, _lv = mi.pop(j)
                    iv, f = ivf.pop(j)
                    stats_b(j, mu, iv, f)

                nc.gpsimd.dma_start(stats_d[:, :], st)

            if loop_n is not None:
                with tc.For_i(0, loop_n, 1,
                              hint_engines=(mybir.EngineType.PE,
                                            mybir.EngineType.Activation,
                                            mybir.EngineType.DVE,
                                            mybir.EngineType.Pool)):
                    for _rep in range(repeat):
                        body()
            else:
                for _rep in range(repeat):
                    body()

    nc.finalize()
    if DEDUP_LDW:
        n = _dedup_ldweights(nc)
        print(f"dedup_ldweights removed {n}")
    return nc


def _dedup_ldweights(nc):
    """Drop InstLdweights whose weights AP is identical to the previous PE
    weight load with only matmuls in between -- the weights are still
    resident in the PE array, and the redundant load costs ~53 ns of serial
    PE time each (bass emits one load per matmul with no reuse detection).
    """
    removed = 0
    for f in nc.m.functions:
        for bb in f.blocks:
            insts = list(bb.instructions)
            keep = []
            last_sig = None
            ok_since = True
            for ins in insts:
                eng = str(getattr(ins, "engine", ""))
                nm = type(ins).__name__
                if eng == "EngineType.PE":
                    if nm == "InstLdweights":
                        sig = str(ins.ins[0])
                        si = ins.sync_info
                        nw = len(si.on_wait) if si else 0
                        if sig == last_sig and ok_since and nw == 0:
                            removed += 1
                            continue
                        last_sig = sig
                        ok_since = True
                    elif nm != "InstMatmult":
                        ok_since = False
                        last_sig = None
                keep.append(ins)
            if len(keep) != len(insts):
                while len(bb.instructions):
                    bb.instructions.pop()
                for ins in keep:
                    bb.instructions.append(ins)
    return removed


def prepare_in_maps(modal_a, modal_b, W1m, b1m, W2m, b2m, W1v, b1v, W2v, b2v):
    w1mT = np.ascontiguousarray(np.asarray(W1m, np.float32).T.astype(_F8))
    w2mT = np.ascontiguousarray(np.asarray(W2m, np.float32).T.astype(_F8))
    w1vT = np.ascontiguousarray(np.asarray(W1v, np.float32).T.astype(_F8))
    w2vT = np.ascontiguousarray(np.asarray(W2v, np.float32).T.astype(_F8))
    bias_pack = np.zeros((P, 4 * JT), np.float32)
    for l, bias in enumerate((b1v, b2v, b1m, b2m)):
        bias_pack[:, l * JT:(l + 1) * JT] = np.asarray(
            bias, np.float32).reshape(JT, P).T

    a_f8 = np.asarray(modal_a, np.float32).astype(_F8)
    b_f32 = np.asarray(modal_b, np.float32)
    b_bf = b_f32.astype(_BF16)
    b2_bf = (b_f32 * b_f32).astype(_BF16)
    in_maps = []
    for c in range(NCORES):
        rows = slice(c * BS, (c + 1) * BS)
        in_maps.append({
            "aT": np.ascontiguousarray(a_f8[rows].T),
            "bT": np.ascontiguousarray(b_bf[rows].T),
            "bT2": np.ascontiguousarray(b2_bf[rows].T),
            "w1mT": w1mT, "w2mT": w2mT, "w1vT": w1vT, "w2vT": w2vT,
            "biases": bias_pack,
        })
    return in_maps


def combine_stats(stats_list):
    cols = np.zeros((7, H), np.float64)
    for st in stats_list:
        st = st.astype(np.float64).reshape(P, JT, NST)
        for c in range(7):
            # feature h = j*128 + p  ->  [JT, P] -> flat
            cols[c] += st[:, :, c].T.reshape(H)
    T0, S1, T1, T2, V, U, S2 = cols

    Ps = T2 - 2.0 * U + V
    mu_mean = S1 / B
    mu_sq_mean = S2 / B
    lld = -0.5 / B * Ps.sum()
    neg_total = -0.5 * (mu_sq_mean @ T0 - 2.0 * (mu_mean @ T1) + T2.sum())
    bound = lld - neg_total / B
    return (np.float32(lld), np.float32(bound))


def kernel(modal_a, modal_b, W1m, b1m, W2m, b2m, W1v, b1v, W2v, b2v):
    if "nc" not in _CACHE:
        _CACHE["nc"] = _build()
    nc = _CACHE["nc"]

    in_maps = prepare_in_maps(modal_a, modal_b, W1m, b1m, W2m, b2m,
                              W1v, b1v, W2v, b2v)
    # One retry: a previously-wedged device surfaces as a runtime error on
    # the first execution and is reset by the failed attempt.
    try:
        res = run_bass_kernel_spmd(nc, in_maps, core_ids=list(range(NCORES)))
    except Exception:
        res = run_bass_kernel_spmd(nc, in_maps, core_ids=list(range(NCORES)))
    return combine_stats([res.results[c]["stats"] for c in range(NCORES)])
